# revision 15
# baseline (speedup 1.0000x reference)
"""Trainium2 Bass kernel: 2-layer GRU (H=128) over 28 timesteps + Linear head.

Reference computation (PyTorch GRUCell semantics, gates r,z,n):
    for t in 28 rows of each 28x28 image:
        h1 = relu(gru1(x_t, h1));  h2 = relu(gru2(h1, h2))
    out = h2 @ w_out.T + b_out

Sharding: pure data parallel, batch 32768 -> 8 cores x 4096.
On-chip layout: transposed [hidden=partition, batch=free]; batch tiled 8x512.

v2 design (software-pipelined cell stream):
  - The 448 GRU-cell evaluations per core (28 t x 2 layers x 8 subtiles) form
    one linear stream; layer-major sweeps make 8 consecutive cells
    independent, so every engine always has ready work.
  - n-gate add folded into PSUM: the r-half rz bank keeps its has_written
    bits after sigmoid consumes it, so the DVE stores t1=(ghn+b)*r into that
    bank and the deferred (lag-2) w_in matmul accumulates gi on top with
    start=False. This removes one [128,512] DVE op per cell.
  - Elementwise rebalanced: DVE does t1 + most of the blend, GpSimd takes
    relu (+ d for L1), ACT does sigmoid/tanh only.
Bias folding:
  - L1: x augmented with ones row; w1aug row 28 carries b_ih1(+b_hh1 for r,z).
  - L2: r/z biases via ScalarE activation bias; b_ih2n via tanh bias.
  - b_hh*n folded into the (ghn + b) * r fused scalar_tensor_tensor.
"""

import json
import os
from contextlib import ExitStack

import ml_dtypes
import numpy as np

import concourse.bass as bass
import concourse.tile as tile
from concourse import mybir
from concourse.bass_utils import run_bass_kernel_spmd

HID = 128
T = 28
C = 28
KAUG = C + 1
NCORES = 8
N_TOTAL = 32768
B_CORE = N_TOTAL // NCORES  # 4096
BF = 512                    # batch tile (matmul free dim / psum bank)
NSUB = B_CORE // BF         # 8
NOUT = 10
LAG = 2                     # cells between t1 store and gi accumulate

F32 = mybir.dt.float32
BF16 = mybir.dt.bfloat16
AF = mybir.ActivationFunctionType
ALU = mybir.AluOpType

# NOTE: GpSimd measured 7.4us per tensor_scalar and poisons DVE throughput
# via SBUF port contention -- keep all elementwise work on the vector engine.

# stash of the last run's perf results for test harness inspection
LAST_RESULT = None


def _split_multi_waits(bir_bytes: bytes) -> bytes:
    """This walrus build rejects instructions carrying >1 sync wait
    ("Too many sync wait commands"). Split extras into standalone
    single-wait EventSemaphore instructions on the same engine, placed
    immediately before -- semantically identical blocking."""
    d = json.loads(bir_bytes)
    ctr = 0
    for fn in d["functions"]:
        for bb in fn["blocks"]:
            out = []
            for inst in bb["instructions"]:
                si = inst.get("sync_info")
                waits = (si or {}).get("on_wait") or []
                if len(waits) > 1:
                    for w in waits[:-1]:
                        ctr += 1
                        out.append({
                            "debug": inst.get("debug", 0),
                            "engine": inst.get("engine"),
                            "ins": [],
                            "outs": [],
                            "name": f"xw-{ctr}",
                            "opcode": "EventSemaphore",
                            "sync_info": {"on_update": [], "on_wait": [w]},
                        })
                    si["on_wait"] = [waits[-1]]
                out.append(inst)
            bb["instructions"] = out
    return json.dumps(d).encode()


def _build_bass() -> bass.Bass:
    nc = bass.Bass()

    x = nc.dram_tensor("x", [(T + 2) // 3, 128, B_CORE], BF16, kind="ExternalInput")
    w1aug_d = nc.dram_tensor("w1aug", [128, 3 * HID], BF16, kind="ExternalInput")
    whh1_d = nc.dram_tensor("whh1T", [HID, 3 * HID], BF16, kind="ExternalInput")
    wih2_d = nc.dram_tensor("wih2T", [HID, 3 * HID], BF16, kind="ExternalInput")
    whh2_d = nc.dram_tensor("whh2T", [HID, 3 * HID], BF16, kind="ExternalInput")
    wout_d = nc.dram_tensor("woutT", [HID, NOUT], BF16, kind="ExternalInput")
    # bias columns: 0=b2r, 1=b2z, 2=b_hh1n, 3=b_hh2n, 4=b_ih2n
    bias_d = nc.dram_tensor("biases", [HID, 5], F32, kind="ExternalInput")
    bout_d = nc.dram_tensor("bout", [NOUT, BF], F32, kind="ExternalInput")
    out_d = nc.dram_tensor("out", [NOUT, B_CORE], F32, kind="ExternalOutput")

    with ExitStack() as ctx:
        tc = ctx.enter_context(tile.TileContext(nc))

        consts = ctx.enter_context(tc.tile_pool(name="consts", bufs=1))
        # PSUM: rz [128,1024] = 2 banks x 3 bufs = 6; gh 1 bank x 2 = 2
        prz = ctx.enter_context(tc.tile_pool(name="prz", bufs=3, space="PSUM"))
        pgh = ctx.enter_context(tc.tile_pool(name="pgh", bufs=2, space="PSUM"))
        spool = ctx.enter_context(tc.tile_pool(name="sp", bufs=3))
        hpool = ctx.enter_context(tc.tile_pool(name="hp", bufs=2))
        opool = ctx.enter_context(tc.tile_pool(name="op", bufs=1))

        w1 = consts.tile([128, 3 * HID], BF16)
        nc.sync.dma_start(out=w1, in_=w1aug_d[:, :])
        wh1 = consts.tile([HID, 3 * HID], BF16)
        nc.sync.dma_start(out=wh1, in_=whh1_d[:, :])
        wi2 = consts.tile([HID, 3 * HID], BF16)
        nc.sync.dma_start(out=wi2, in_=wih2_d[:, :])
        wh2 = consts.tile([HID, 3 * HID], BF16)
        nc.sync.dma_start(out=wh2, in_=whh2_d[:, :])
        wo = consts.tile([HID, NOUT], BF16)
        nc.sync.dma_start(out=wo, in_=wout_d[:, :])
        bs = consts.tile([HID, 5], F32)
        nc.sync.dma_start(out=bs, in_=bias_d[:, :])
        bo = consts.tile([NOUT, BF], F32)
        nc.sync.dma_start(out=bo, in_=bout_d[:, :])

        xg = []
        for g in range((T + 2) // 3):
            xt_ = consts.tile([128, B_CORE], BF16, tag=f"xg_{g}", name=f"xg_{g}")
            nc.sync.dma_start(out=xt_, in_=x[g, :, :])
            xg.append(xt_)

        # hidden state kept in quad-wide tiles [128, 4*BF]: quad q covers
        # subtiles 4q..4q+3 so the blend processes 4 cells per DVE op.
        QW = 4
        hcur = [{}, {}]
        for l in (0, 1):
            for q in range(NSUB // QW):
                hp_ = hpool.tile([HID, QW * BF], BF16, tag=f"h{l + 1}_{q}",
                                 name=f"h{l + 1}i_{q}")
                nc.vector.memset(hp_, 0.0)
                hcur[l][q] = hp_

        def h_slice(l, s):
            return hcur[l][s // QW][:, (s % QW) * BF:(s % QW) * BF + BF]

        def cell_params(t, layer, s):
            """Weight/bias/rhs lookup for one cell."""
            if layer == 0:
                g, j = divmod(t, 3)
                xa = xg[g][32 * j:32 * j + KAUG, s * BF:(s + 1) * BF]
                return dict(w_ih=w1, wbase=32 * j, kin=KAUG, w_hh=wh1,
                            xa=xa, sig_bias=None, t1_bias=bs[:, 2:3],
                            tanh_bias=0.0)
            return dict(w_ih=wi2, wbase=0, kin=HID, w_hh=wh2,
                        xa=h_slice(0, s), sig_bias=(bs[:, 0:1], bs[:, 1:2]),
                        t1_bias=bs[:, 3:4], tanh_bias=bs[:, 4:5])

        # cell stream: for t: layer 0 sweep s=0..7, then layer 1 sweep
        cells = [(t, l, s) for t in range(T) for l in (0, 1)
                 for s in range(NSUB)]
        state = {}  # idx -> dict with in-flight tiles

        pstate = {}  # quad index -> dict with quad-wide tiles

        def emit_mm_early(i):
            """rz + gh matmuls for cell i."""
            t, layer, s = cells[i]
            p = cell_params(t, layer, s)
            h_prev = h_slice(layer, s)
            if i % QW == 0:
                pstate[i // QW] = dict(h_prev=hcur[layer][s // QW])
            rz = prz.tile([HID, 2 * BF], F32, tag="rz")
            nc.tensor.matmul(rz[:, 0:BF],
                             p["w_ih"][p["wbase"]:p["wbase"] + p["kin"], 0:HID],
                             p["xa"], start=True, stop=False)
            nc.tensor.matmul(rz[:, 0:BF], p["w_hh"][:, 0:HID], h_prev,
                             start=False, stop=True)
            nc.tensor.matmul(rz[:, BF:2 * BF],
                             p["w_ih"][p["wbase"]:p["wbase"] + p["kin"],
                                       HID:2 * HID],
                             p["xa"], start=True, stop=False)
            nc.tensor.matmul(rz[:, BF:2 * BF], p["w_hh"][:, HID:2 * HID],
                             h_prev, start=False, stop=True)
            gh = pgh.tile([HID, BF], F32, tag="gh")
            nc.tensor.matmul(gh, p["w_hh"][:, 2 * HID:3 * HID], h_prev,
                             start=True, stop=True)
            state[i] = dict(rz=rz, gh=gh, p=p)

        def emit_mm_gi(i):
            """Deferred w_in matmul accumulating gi onto the t1 psum."""
            st = state[i]
            rz, p = st["rz"], st["p"]
            nc.tensor.matmul(rz[:, 0:BF],
                             p["w_ih"][p["wbase"]:p["wbase"] + p["kin"],
                                       2 * HID:3 * HID],
                             p["xa"], start=False, stop=True,
                             skip_group_check=True)

        def emit_act_sig(i):
            """Sigmoid r|z for cell i into its slot of the quad rzs tile."""
            st = state[i]
            rz, p = st["rz"], st["p"]
            if i % QW == 0:
                pstate[i // QW]["rzs"] = spool.tile([HID, 2 * QW * BF], BF16,
                                                    tag="rzs",
                                                    name=f"rzs_{i // QW}")
            rzs = pstate[i // QW]["rzs"]
            off = (i % QW) * 2 * BF
            if p["sig_bias"] is None:
                nc.scalar.activation(rzs[:, off:off + 2 * BF], rz, AF.Sigmoid)
            else:
                nc.scalar.activation(rzs[:, off:off + BF], rz[:, 0:BF],
                                     AF.Sigmoid, bias=p["sig_bias"][0])
                nc.scalar.activation(rzs[:, off + BF:off + 2 * BF],
                                     rz[:, BF:2 * BF], AF.Sigmoid,
                                     bias=p["sig_bias"][1])

        def emit_t1(i):
            # t1 = (ghn + b_hhn) * r stored over the (dead) r pre-activation;
            # its has_written bits stay set so the deferred gi matmul can
            # accumulate on top with start=False.
            st = state[i]
            off = (i % QW) * 2 * BF
            nc.vector.scalar_tensor_tensor(st["rz"][:, 0:BF], st["gh"],
                                           st["p"]["t1_bias"],
                                           pstate[i // QW]["rzs"][:, off:off + BF],
                                           op0=ALU.add, op1=ALU.mult)

        def emit_act_tanh(i):
            st = state[i]
            if i % QW == 0:
                pstate[i // QW]["nsb"] = spool.tile([HID, QW * BF], BF16,
                                                    tag="nsb",
                                                    name=f"nsb_{i // QW}")
            nsb = pstate[i // QW]["nsb"]
            off = (i % QW) * BF
            nc.scalar.activation(nsb[:, off:off + BF], st["rz"][:, 0:BF],
                                 AF.Tanh, bias=st["p"]["tanh_bias"])

        def emit_blend_quad(i):
            """h' = relu(n + z*(h-n)) for the quad ending at cell i."""
            t, layer, s = cells[i]
            q = s // QW
            ps = pstate.pop(i // QW)
            for k in range(QW):
                state.pop(i - k)
            rzs, h_prev, nsb = ps["rzs"], ps["h_prev"], ps["nsb"]
            # z-slots of the quad: [128, QW, 512] strided view of rzs
            zp = rzs.rearrange("p (a b) -> p a b", b=BF)[:, 1::2, :]
            d = spool.tile([HID, QW * BF], BF16, tag="d")
            nc.vector.tensor_tensor(d, h_prev, nsb, op=ALU.subtract)
            zd = spool.tile([HID, QW * BF], BF16, tag="zd")
            nc.vector.tensor_tensor(zd, zp, d, op=ALU.mult)
            hpre = spool.tile([HID, QW * BF], BF16, tag="hpre")
            nc.vector.tensor_tensor(hpre, zd, nsb, op=ALU.add)
            hn = hpool.tile([HID, QW * BF], BF16, tag=f"h{layer + 1}_{q}",
                            name=f"h{layer + 1}_{q}_t{t}")
            nc.vector.tensor_scalar_max(hn, hpre, 0.0)
            hcur[layer][q] = hn

        # Software pipeline, LAG cells deep: per iteration the engine queues
        # only see work whose inputs were produced >= 1 iteration earlier.
        # T: gi(i-2), early(i); A: tanh(i-2), sig(i); V: blend(quad), t1(i).
        N = len(cells)
        for i in range(N + LAG):
            if i >= LAG:
                emit_mm_gi(i - LAG)
                emit_act_tanh(i - LAG)
                if (i - LAG) % QW == QW - 1:
                    emit_blend_quad(i - LAG)
            if i < N:
                emit_mm_early(i)
                emit_act_sig(i)
                emit_t1(i)

        ob = opool.tile([NOUT, B_CORE], F32, tag="ob")
        for s in range(NSUB):
            po = prz.tile([NOUT, BF], F32, tag="rz")
            nc.tensor.matmul(po, wo, h_slice(1, s), start=True, stop=True)
            nc.vector.tensor_tensor(ob[:, s * BF:(s + 1) * BF], po, bo,
                                    op=ALU.add)
        nc.scalar.dma_start(out=out_d[:, :], in_=ob)

    return nc


def _prep_inputs(x, w_ih1, w_hh1, b_ih1, b_hh1, w_ih2, w_hh2, b_ih2, b_hh2,
                 w_out, b_out):
    """Host-side reshape/transpose/cast + per-core sharding."""
    n = N_TOTAL
    xs = np.asarray(x, np.float32).reshape(n, T, C)       # channel dim is 1
    xt = np.transpose(xs, (1, 2, 0))                      # [T, C, n]
    xg = np.zeros(((T + 2) // 3, 128, n), np.float32)
    for t in range(T):
        g, j = divmod(t, 3)
        xg[g, 32 * j:32 * j + C, :] = xt[t]
        xg[g, 32 * j + C, :] = 1.0
    xg16 = xg.astype(ml_dtypes.bfloat16)

    w_ih1 = np.asarray(w_ih1, np.float32)
    w_hh1 = np.asarray(w_hh1, np.float32)
    b_ih1 = np.asarray(b_ih1, np.float32)
    b_hh1 = np.asarray(b_hh1, np.float32)
    w_ih2 = np.asarray(w_ih2, np.float32)
    w_hh2 = np.asarray(w_hh2, np.float32)
    b_ih2 = np.asarray(b_ih2, np.float32)
    b_hh2 = np.asarray(b_hh2, np.float32)
    w_out = np.asarray(w_out, np.float32)
    b_out = np.asarray(b_out, np.float32)

    H = HID
    w1aug = np.zeros((128, 3 * H), np.float32)
    bias_row = np.concatenate([
        b_ih1[0:H] + b_hh1[0:H],          # r: both biases
        b_ih1[H:2 * H] + b_hh1[H:2 * H],  # z: both biases
        b_ih1[2 * H:3 * H],               # n: input-side bias only
    ])
    for j in range(4):
        w1aug[32 * j:32 * j + C, :] = w_ih1.T
        w1aug[32 * j + C, :] = bias_row
    # rows 32j+29..32j+31 stay zero (padding lanes in x tiles)

    biases = np.stack([
        b_ih2[0:H] + b_hh2[0:H],
        b_ih2[H:2 * H] + b_hh2[H:2 * H],
        b_hh1[2 * H:3 * H],
        b_hh2[2 * H:3 * H],
        b_ih2[2 * H:3 * H],
    ], axis=1).astype(np.float32)         # [H, 5]

    common = {
        "w1aug": np.ascontiguousarray(w1aug.astype(ml_dtypes.bfloat16)),
        "whh1T": np.ascontiguousarray(w_hh1.T.astype(ml_dtypes.bfloat16)),
        "wih2T": np.ascontiguousarray(w_ih2.T.astype(ml_dtypes.bfloat16)),
        "whh2T": np.ascontiguousarray(w_hh2.T.astype(ml_dtypes.bfloat16)),
        "woutT": np.ascontiguousarray(w_out.T.astype(ml_dtypes.bfloat16)),
        "biases": np.ascontiguousarray(biases),
        "bout": np.ascontiguousarray(
            np.broadcast_to(b_out.reshape(NOUT, 1), (NOUT, BF)).astype(np.float32)),
    }
    in_maps = []
    for c in range(NCORES):
        m = dict(common)
        m["x"] = np.ascontiguousarray(xg16[:, :, c * B_CORE:(c + 1) * B_CORE])
        in_maps.append(m)
    return in_maps


def kernel(**inputs):
    global LAST_RESULT
    nc = _build_bass()
    edited = _split_multi_waits(nc.to_json_bytes())
    nc.to_json_bytes = lambda: edited
    in_maps = _prep_inputs(**inputs)
    trace = bool(int(os.environ.get("BASS_TRACE", "0")))
    res = run_bass_kernel_spmd(nc, in_maps, core_ids=list(range(NCORES)),
                               trace=trace)
    LAST_RESULT = res
    outs = [r["out"] for r in res.results]          # each [NOUT, B_CORE] f32
    full = np.concatenate(outs, axis=1)             # [NOUT, N_TOTAL]
    return np.ascontiguousarray(full.T).astype(np.float32)
